# revision 1
# baseline (speedup 1.0000x reference)
"""Multi-head attention (B=1, S=4096, D=768, 12 heads) on 8 trn2 cores.

Sharding: tensor-parallel by heads, balanced with sequence splits.
Core c owns: head A = c (all 4096 query rows) and head B = 8 + c//2
(query-row half c%2).  Each core computes q/k/v for its two heads, full
S x S attention for its share, and its heads' partial contribution to
the output projection (row-parallel split of w_proj).  The host sums
the per-core projection partials and adds the bias.

Device layout: d-on-partitions ("transposed") everywhere.  Scores are
computed as S^T[t, s] = K^T.T @ Q^T per 128-key chunk; exp on ScalarE
(logits are bounded, no max subtraction); the AV matmul uses a
[V | ones] stationary operand so rows 0-63 of PSUM accumulate O^T and
rows 64-127 the softmax denominator in the same pass.

The attention loop runs "sweeps" that each process TWO units at once —
one on PE row-groups 0-1 (SBUF partitions 0-63) and one on row-groups
2-3 (partitions 64-127) — so the K=64 score matmuls run pairwise
concurrent on the PE array.  Head A's second half is paired with itself
via a partition-shifted duplicate of Q^T/K^T.  q/k/v generation and the
output projection are woven into the sweeps to keep PE dense (HAM).
All matmuls are bf16 with fp32 PSUM accumulation.
"""

import numpy as np
import ml_dtypes

import concourse.bass as bass
import concourse.mybir as mybir
import concourse.tile as tile
from concourse import bacc
from concourse.bass_utils import run_bass_kernel_spmd

BF16 = mybir.dt.bfloat16
F32 = mybir.dt.float32
ts = bass.ts
ds = bass.ds

S = 4096
D = 768
NH = 12
HD = 64
NCORES = 8
SU = 2048          # rows per unit
PO = D // 128      # 6 e-chunks
NT = S // 128      # 32 key chunks
SCALE = HD ** -0.5
SPL = 512          # exp split point: ScalarE does [0:SPL] (= pt_L), DVE the rest (= pt_R)

_CACHE: dict = {}

# --- custom DVE exp op: out_uint16 = bf16 bits of 2^((x - 64)/128) ---------
# Magic-constant round to the 128-grid + quadratic mantissa correction,
# emitted through the fp32->uint16 value cast.  The -64 window shift (a
# global 2^-0.5 factor on all exp values) cancels in the softmax
# normalization; the ScalarE branch matches it via the activation bias.
EXP_M = 1.5 * 2**30
EXP_Q0 = 16180.991964579287
EXP_Q1 = 0.9950478871994926
EXP_Q2 = 0.0026875086476569427
EXP_SCALE = float(np.log(2) / 128.0)
EXP_BIAS = float(-np.log(2) / 2.0)
LOG2E_128 = float(128.0 / np.log(2))


def _expb_ref(in0, in1, s0, s1, imm2):
    f32 = np.float32
    a = (in0.astype(f32) + f32(s0)).astype(f32)
    u = (a - f32(s0)).astype(f32)
    z = (in0.astype(f32) - u).astype(f32)
    m2 = (((z * f32(s1)).astype(f32) + f32(imm2)).astype(f32) * z).astype(f32)
    return ((u + m2).astype(f32) + in1.astype(f32)).astype(f32)


def _expb_op():
    from concourse import dve_ops
    from concourse.dve_spec import Spec, Src0, C0, C1, C2, C3, lower, _spill_c3_to_src1
    from concourse.dve_uop import DveOpSpec

    for op in dve_ops.OPS:
        if op.name == "EXPB_ANT":
            return op
    a = Src0 + C0
    u = a - C0
    z = Src0 - u
    m2 = (z * C1 + C2) * z
    body = _spill_c3_to_src1((u + m2) + C3)
    spec = Spec(body=body, reference=_expb_ref)
    row = dve_ops._CUSTOM_DVE_ROW_BASE + len(dve_ops.OPS)
    dve_ops._SUB_OPCODE_FOR_NAME["EXPB_ANT"] = row
    shas = {}
    for ver in ("v3", "v4"):
        try:
            uops = lower(spec, ver=ver)
            shas[ver] = DveOpSpec(
                name="EXPB_ANT", opcode=row, uops=uops, rd1_en=True
            ).sha(ver)
        except Exception:
            pass
    op = dve_ops.DveOp("EXPB_ANT", spec, subdim=False, uops_sha=shas)
    dve_ops.OPS.append(op)
    dve_ops.CUSTOM_DVE_SPECS["EXPB_ANT"] = spec
    return op


def _emit(nc):
    xT = nc.dram_tensor("xT", [D, S], BF16, kind="ExternalInput")
    xq = nc.dram_tensor("xq", [D, SU], BF16, kind="ExternalInput")
    wq = nc.dram_tensor("wq", [D, 128], BF16, kind="ExternalInput")
    wk = nc.dram_tensor("wk", [D, 128], BF16, kind="ExternalInput")
    wv = nc.dram_tensor("wv", [D, 128], BF16, kind="ExternalInput")
    wp = nc.dram_tensor("wp", [64, 2, D], BF16, kind="ExternalInput")
    yTa = nc.dram_tensor("yTa", [D, S], BF16, kind="ExternalOutput")
    yTb = nc.dram_tensor("yTb", [D, SU], BF16, kind="ExternalOutput")

    xT_v = xT.ap().rearrange("(po pi) s -> pi po s", pi=128)
    xq_v = xq.ap().rearrange("(po pi) s -> pi po s", pi=128)
    wq_v = wq.ap().rearrange("(po pi) o -> pi po o", pi=128)
    wk_v = wk.ap().rearrange("(po pi) o -> pi po o", pi=128)
    wv_v = wv.ap().rearrange("(po pi) o -> pi po o", pi=128)
    yTa_v = yTa.ap().rearrange("(po pi) s -> pi po s", pi=128)
    yTb_v = yTb.ap().rearrange("(po pi) s -> pi po s", pi=128)

    with tile.TileContext(nc) as tc:
        with (
            tc.tile_pool(name="persist", bufs=1) as pp,
            tc.tile_pool(name="work", bufs=4) as wkp,
            tc.tile_pool(name="ps", bufs=2, space="PSUM") as psp,
        ):
            ptp = otp = rbp = ytp = wkp
            scp = avp = pjp = psp
            # ---------------- persistent SBUF tensors ----------------
            xT_sb = pp.tile([128, PO, S], BF16)
            xq_sb = pp.tile([128, PO, SU], BF16)
            wq_sb = pp.tile([128, PO, 128], BF16)
            wk_sb = pp.tile([128, PO, 128], BF16)
            wv_sb = pp.tile([128, PO, 128], BF16)
            wp_sb = pp.tile([64, 2, D], BF16)
            QT_sb = pp.tile([128, S], BF16)      # 0:64 A (full S); 64:128 B (cols 0:SU) + A-dup (cols 3072:4096)
            KT_sb = pp.tile([128, S], BF16)      # 0:64 A, 64:128 B
            KT2_sb = pp.tile([128, S], BF16)     # 64:128 = copy of A rows (for self-pairing)
            VT_sb = pp.tile([128, S], BF16)      # V^T staging for the DMA transpose
            V_sb = pp.tile([128, NT, 256], BF16)  # [V_A |ones| V_B |ones]

            nc.sync.dma_start(wq_sb[:], wq_v)
            nc.gpsimd.dma_start(wk_sb[:], wk_v)
            nc.gpsimd.dma_start(wv_sb[:], wv_v)
            for n in range(8):
                nc.sync.dma_start(xT_sb[:, :, ts(n, 512)], xT_v[:, :, ts(n, 512)])
                if n < 4:
                    nc.gpsimd.dma_start(xq_sb[:, :, ts(n, 512)], xq_v[:, :, ts(n, 512)])
            nc.gpsimd.dma_start(wp_sb[:], wp.ap())
            q0_sb = pp.tile([128, 1], F32)
            bias_sb = pp.tile([128, 1], F32)
            nc.vector.memset(V_sb[:, :, 64:128], 1.0)
            nc.vector.memset(V_sb[:, :, 192:256], 1.0)
            # HAM pre-warm: ~64 dense matmuls while input DMAs stream, so the
            # PE clock is at 2.4 GHz when real work starts
            warm_sb = pp.tile([128, 128], BF16)
            nc.vector.memset(warm_sb[:], 0.0)
            warm_ps = pjp.tile([128, 512], F32, tag="pj", name="warm_ps", bufs=1)
            for i in range(128):
                nc.tensor.matmul(
                    warm_ps[:, 0:128], lhsT=warm_sb[:], rhs=warm_sb[:],
                    start=(i == 0), stop=(i == 127),
                )
            nc.vector.memset(q0_sb[:], EXP_Q0)
            nc.vector.memset(bias_sb[:], EXP_BIAS)
            expb = _expb_op()

            # ---------------- q/k/v projections (woven into sweep 0) -------
            pfx_alt = [0]

            def pfx_copy(dst, srcv):
                pfx_alt[0] ^= 1
                if pfx_alt[0]:
                    nc.vector.tensor_copy(dst, srcv)
                else:
                    nc.scalar.copy(dst, srcv)

            def emit_qt_block(n, qa_pt, qb_pt):
                qa_ps = qa_pt[0].tile([128, 512], F32, tag=qa_pt[1], name="qa_ps", bufs=3 if qa_pt[1] == "av" else 1)
                qb_ps = (
                    qb_pt[0].tile([128, 512], F32, tag=qb_pt[1],
                                  name="qb_ps", bufs=3 if qb_pt[1] == "av" else 1)
                    if n < 4 else None
                )
                for po in range(PO):
                    nc.tensor.matmul(
                        qa_ps[0:64, 0:512],
                        lhsT=wq_sb[:, po, 0:64],
                        rhs=xT_sb[:, po, ts(n, 512)],
                        start=(po == 0),
                        stop=(po == PO - 1),
                    )
                    if qb_ps is not None:
                        nc.tensor.matmul(
                            qb_ps[64:128, 0:512],
                            lhsT=wq_sb[:, po, 64:128],
                            rhs=xq_sb[:, po, ts(n, 512)],
                            start=(po == 0),
                            stop=(po == PO - 1),
                        )
                pfx_copy(QT_sb[0:64, ts(n, 512)], qa_ps[0:64, 0:512])
                if qb_ps is not None:
                    pfx_copy(QT_sb[64:128, ts(n, 512)], qb_ps[64:128, 0:512])

            def emit_kt_block(n, pool, tag):
                k_ps = pool.tile([128, 512], F32, tag=tag, name="k_ps", bufs=3 if tag == "av" else 1)
                for po in range(PO):
                    nc.tensor.matmul(
                        k_ps[:, 0:512],
                        lhsT=wk_sb[:, po, :],
                        rhs=xT_sb[:, po, ts(n, 512)],
                        start=(po == 0),
                        stop=(po == PO - 1),
                    )
                pfx_copy(KT_sb[:, ts(n, 512)], k_ps[:, 0:512])

            def emit_vt_block(n, pool, tag):
                v_ps = pool.tile([128, 512], F32, tag=tag, name="v_ps", bufs=3 if tag == "av" else 1)
                for po in range(PO):
                    nc.tensor.matmul(
                        v_ps[:, 0:512],
                        lhsT=wv_sb[:, po, :],
                        rhs=xT_sb[:, po, ts(n, 512)],
                        start=(po == 0),
                        stop=(po == PO - 1),
                    )
                pfx_copy(VT_sb[:, ts(n, 512)], v_ps[:, 0:512])
                vv = V_sb[:, 4 * n : 4 * n + 4, :]
                nc.scalar.dma_start_transpose(vv[:, :, 0:64], VT_sb[0:64, ts(n, 512)])
                nc.scalar.dma_start_transpose(vv[:, :, 128:192], VT_sb[64:128, ts(n, 512)])

            # ---------------- attention sweeps ----------------
            # unit specs: (ot_key, vbase, wp_idx, (ydst, ycolbase))
            USPEC = {
                "u0": (0, 0, yTa_v, 0),
                "u1": (0, 0, yTa_v, SU),
                "u2": (128, 1, yTb_v, 0),
            }
            # sweeps: (L, R) sides: (unit, kt_tile, slot, qt_abs_col, ot_local_col)
            sweeps = []
            for sb in range(4):
                sweeps.append(
                    (
                        ("u0", KT_sb, 0, sb * 512, sb * 512),
                        ("u2", KT_sb, 64, sb * 512, sb * 512),
                    )
                )
            for sb in range(2):
                sweeps.append(
                    (
                        ("u1", KT_sb, 0, SU + sb * 512, sb * 512),
                        ("u1", KT2_sb, 64, SU + 1024 + sb * 512, 1024 + sb * 512),
                    )
                )

            ot_tiles = {
                "u0": otp.tile([64, SU], BF16, tag="ot", name="ot_u0", bufs=3),
                "u1": otp.tile([64, SU], BF16, tag="ot", name="ot_u1", bufs=3),
                "u2": otp.tile([64, SU], BF16, tag="ot", name="ot_u2", bufs=3),
            }
            proj_q = []
            tail_alt = [0]
            cp_alt = [0]

            def emit_proj_chunk(u, oe, col, tail=False):
                vbase_unused, wpi, ydst, ybase = USPEC[u]
                # in the tail the av banks are free again: alternate tags so
                # four banks rotate and the drain pipelines
                tail_alt[0] ^= 1
                tag, pool = ("av", avp) if (tail and tail_alt[0]) else ("pj", pjp)
                pj = pool.tile([128, 512], F32, tag=tag, name="pj", bufs=3 if tag == "av" else 1)
                nc.tensor.matmul(
                    pj[:],
                    lhsT=wp_sb[:, wpi, ts(oe, 128)],
                    rhs=ot_tiles[u][0:64, ds(col, 512)],
                    start=True,
                    stop=True,
                )
                yt = ytp.tile([128, 512], BF16, tag="yt", name="yt", bufs=6)
                cp_alt[0] = (cp_alt[0] + 1) % (2 if tail else 3)
                if cp_alt[0] == 0:
                    nc.scalar.copy(yt[:], pj[:])
                else:
                    nc.vector.tensor_copy(yt[:], pj[:])
                oq = nc.sync if (tail or tail_alt[0]) else nc.gpsimd
                oq.dma_start(ydst[:, oe, ds(col + ybase, 512)], yt[:])

            def pump(k):
                for _ in range(k):
                    if proj_q:
                        proj_q.pop(0)()

            emit_qt_block(0, (avp, "av"), (avp, "av"))
            emit_kt_block(0, avp, "av")
            emit_vt_block(0, avp, "av")
            for si, (Lside, Rside) in enumerate(sweeps):
                if si == 1:
                    # duplicates for head-A self-pairing (after KT/QT complete)
                    nc.gpsimd.dma_start(KT2_sb[64:128, :], KT_sb[0:64, :])
                    nc.gpsimd.dma_start(
                        QT_sb[64:128, SU + 1024 : S], QT_sb[0:64, SU + 1024 : S]
                    )
                avL = avp.tile([128, 512], F32, tag="av", name="avL", bufs=3)
                avR = avp.tile([128, 512], F32, tag="av", name="avR", bufs=3)

                def emit_av(t, ptL, ptR):
                    for u, av, ptt in ((Lside[0], avL, ptL), (Rside[0], avR, ptR)):
                        vbase = USPEC[u][0]
                        nc.tensor.matmul(
                            av,
                            lhsT=V_sb[:, t, vbase : vbase + 128],
                            rhs=ptt[:, 0:512],
                            start=(t == 0),
                            stop=(t == NT - 1),
                        )

                pend = []
                for t in range(NT):
                    if si == 0 and t % 4 < 3 and t // 4 < 7:
                        m = t // 4
                        if t % 4 == 0:
                            emit_vt_block(m + 1, pjp, "pj")
                        elif t % 4 == 1:
                            emit_kt_block(m + 1, pjp, "pj")
                        else:
                            # qa-only blocks first (no xq dependency); the
                            # xq-dependent blocks late, once xq has streamed in
                            emit_qt_block(m + 4 if m < 4 else m - 3,
                                          (pjp, "pj"), (pjp, "pj"))
                    scL = scp.tile([128, 512], F32, tag="scL", name="scL")
                    scR = scp.tile([128, 512], F32, tag="scR", name="scR")
                    for (u, ktt, slot, qcol, _ocol), sct in ((Lside, scL), (Rside, scR)):
                        nc.tensor.matmul(
                            sct[:, 0:512],
                            lhsT=ktt[slot : slot + 64, ts(t, 128)],
                            rhs=QT_sb[slot : slot + 64, ds(qcol, 512)],
                            start=True,
                            stop=True,
                        )
                    ptL = ptp.tile([128, 512], BF16, tag="ptL", name="ptL", bufs=4)
                    ptR = ptp.tile([128, 512], BF16, tag="ptR", name="ptR", bufs=4)
                    nc.scalar.activation(
                        ptL[:, 0:512],
                        scL[:, 0:512],
                        mybir.ActivationFunctionType.Exp,
                        bias=bias_sb[:],
                        scale=EXP_SCALE,
                    )
                    nc.vector._custom_dve(
                        expb,
                        out=ptR[:, 0:512].bitcast(mybir.dt.uint16),
                        in0=scR[:, 0:512],
                        in1=q0_sb[:],
                        s0=EXP_M,
                        s1=EXP_Q2,
                        imm2=EXP_Q1,
                    )
                    pump(1)
                    # AV matmuls run TWO iterations behind the score pair: by
                    # then their exp inputs are long done, so every PE
                    # instruction is dep-free at issue time (throughput-bound
                    # instead of exp-latency-bound), and the score pair stays
                    # adjacent in the PE stream (concurrent K=64 halves)
                    if len(pend) == 2:
                        emit_av(*pend.pop(0))
                    pend.append((t, ptL, ptR))
                for p in pend:
                    emit_av(*p)
                # drain sweeps: normalize O^T, queue proj work
                for (u, _ktt, _slot, _qcol, ocol), av in ((Lside, avL), (Rside, avR)):
                    rhi = rbp.tile([128, 512], F32, tag="rhi", name="rhi", bufs=3)
                    rlo = rbp.tile([64, 512], F32, tag="rlo", name="rlo", bufs=3)
                    # custom-DVE ops require base_partition 0: run on the full
                    # tile; rows 0:64 (1/O^T) are computed but never read
                    nc.vector.reciprocal_approx_fast(rhi[:], av[:])
                    nc.sync.dma_start(rlo[:], rhi[64:128, :])
                    nc.vector.tensor_mul(
                        ot_tiles[u][0:64, ds(ocol, 512)], av[0:64, :], rlo[:]
                    )
                    for oe in range(PO):
                        proj_q.append(
                            lambda tail=False, u=u, oe=oe, col=ocol: emit_proj_chunk(
                                u, oe, col, tail
                            )
                        )
            tw = scp.tile([128, 512], F32, tag="scL", name="tw")
            for i in range(32):
                nc.tensor.matmul(
                    tw[:, 0:128], lhsT=warm_sb[:], rhs=warm_sb[:],
                    start=(i == 0), stop=(i == 31),
                )
            while proj_q:
                proj_q.pop(0)(tail=True)

    nc.compile()
    return nc


def _build():
    if "nc" not in _CACHE:
        nc = bacc.Bacc(None, target_bir_lowering=False, debug=False)
        _CACHE["nc"] = _emit(nc)
    return _CACHE["nc"]


def _prep_inputs(x, w_qkv, w_proj):
    bf = ml_dtypes.bfloat16
    xs = np.ascontiguousarray(x.reshape(S, D).T).astype(bf)  # [D, S]
    in_maps = []
    for c in range(NCORES):
        ha = c
        hb = 8 + c // 2
        bh = c % 2
        rows_q = lambda h: w_qkv[h * HD : (h + 1) * HD, :]
        rows_k = lambda h: w_qkv[D + h * HD : D + (h + 1) * HD, :]
        rows_v = lambda h: w_qkv[2 * D + h * HD : 2 * D + (h + 1) * HD, :]
        qs = SCALE * LOG2E_128
        wq_c = np.concatenate([rows_q(ha) * qs, rows_q(hb) * qs], 0).T
        wk_c = np.concatenate([rows_k(ha), rows_k(hb)], 0).T
        wv_c = np.concatenate([rows_v(ha), rows_v(hb)], 0).T
        wp_c = np.stack(
            [w_proj[:, ha * HD : (ha + 1) * HD].T, w_proj[:, hb * HD : (hb + 1) * HD].T],
            axis=1,
        )  # [64, 2, D]
        in_maps.append(
            {
                "xT": xs,
                "xq": np.ascontiguousarray(xs[:, bh * SU : (bh + 1) * SU]),
                "wq": np.ascontiguousarray(wq_c).astype(bf),
                "wk": np.ascontiguousarray(wk_c).astype(bf),
                "wv": np.ascontiguousarray(wv_c).astype(bf),
                "wp": np.ascontiguousarray(wp_c).astype(bf),
            }
        )
    return in_maps


def _combine(results, b_proj):
    yT = np.zeros((D, S), np.float32)
    for c in range(NCORES):
        yT += results[c]["yTa"].astype(np.float32)
        bh = c % 2
        yT[:, bh * SU : (bh + 1) * SU] += results[c]["yTb"].astype(np.float32)
    y = yT.T + b_proj.astype(np.float32)[None, :]
    return y.reshape(1, 64, 64, D).astype(np.float32)


def kernel(x, w_qkv, w_proj, b_proj, _trace=False, _trace_kwargs=None):
    x = np.asarray(x, np.float32)
    w_qkv = np.asarray(w_qkv, np.float32)
    w_proj = np.asarray(w_proj, np.float32)
    b_proj = np.asarray(b_proj, np.float32)

    nc = _build()
    in_maps = _prep_inputs(x, w_qkv, w_proj)
    res = run_bass_kernel_spmd(
        nc, in_maps, core_ids=list(range(NCORES)), trace=_trace,
        **(_trace_kwargs or {}),
    )
    out = _combine(res.results, b_proj)
    if _trace:
        return out, res
    return out



# revision 13
# speedup vs baseline: 1.1683x; 1.1683x over previous
"""Multi-head attention (B=1, S=4096, D=768, 12 heads) on 8 trn2 cores.

Sharding: tensor-parallel by heads, balanced with sequence splits.
Core c owns: head A = c (all 4096 query rows) and head B = 8 + c//2
(query-row half c%2).  Each core computes q/k/v for its two heads, full
S x S attention for its share, and its heads' partial contribution to
the output projection (row-parallel split of w_proj).  The host sums
the per-core projection partials and adds the bias.

Device layout: d-on-partitions ("transposed") everywhere.  Scores are
computed as S^T[t, s] = K^T.T @ Q^T per 128-key chunk; exp on ScalarE
(logits are bounded, no max subtraction); the AV matmul uses a
[V | ones] stationary operand so rows 0-63 of PSUM accumulate O^T and
rows 64-127 the softmax denominator in the same pass.

The attention loop runs "sweeps" that each process TWO units at once -
one on PE row-groups 0-1 (SBUF partitions 0-63) and one on row-groups
2-3 (partitions 64-127) - so the K=64 score matmuls run pairwise
concurrent on the PE array.  Head A's second half is paired with itself
via a partition-shifted duplicate of Q^T/K^T.  Inner loop is batched in
pairs of key-chunks so the AV accumulations issue as same-PSUM-bank
runs (avoids the per-matmul bank-switch micro-idle).  Q projections for
the two heads run as column-tiled concurrent pairs sharing one PSUM
bank.  All input DMA is chunk-major fully-contiguous.  All matmuls are
bf16 with fp32 PSUM accumulation.
"""

import numpy as np
import ml_dtypes

import concourse.bass as bass
import concourse.mybir as mybir
import concourse.tile as tile
from concourse import bacc
from concourse.bass_utils import run_bass_kernel_spmd

BF16 = mybir.dt.bfloat16
F32 = mybir.dt.float32
ts = bass.ts
ds = bass.ds

S = 4096
D = 768
NH = 12
HD = 64
NCORES = 8
SU = 2048          # rows per unit
PO = D // 128      # 6 e-chunks
NT = S // 128      # 32 key chunks
NC8 = S // 512     # 8 column chunks
SCALE = HD ** -0.5

_CACHE: dict = {}

# --- custom DVE exp op: out_uint16 = bf16 bits of 2^((x - 64)/128) ---------
# Magic-constant round to the 128-grid + quadratic mantissa correction,
# emitted through the fp32->uint16 value cast.  The -64 window shift (a
# global 2^-0.5 factor on all exp values) cancels in the softmax
# normalization; the ScalarE branch matches it via the activation bias.
EXP_M = 1.5 * 2**30
EXP_Q0 = 16180.991964579287
EXP_Q1 = 0.9950478871994926
EXP_Q2 = 0.0026875086476569427
EXP_SCALE = float(np.log(2) / 128.0)
EXP_BIAS = float(-np.log(2) / 2.0)
LOG2E_128 = float(128.0 / np.log(2))


def _expb_ref(in0, in1, s0, s1, imm2):
    f32 = np.float32
    a = (in0.astype(f32) + f32(s0)).astype(f32)
    u = (a - f32(s0)).astype(f32)
    z = (in0.astype(f32) - u).astype(f32)
    m2 = (((z * f32(s1)).astype(f32) + f32(imm2)).astype(f32) * z).astype(f32)
    return ((u + m2).astype(f32) + in1.astype(f32)).astype(f32)


def _expb_op():
    from concourse import dve_ops
    from concourse.dve_spec import Spec, Src0, C0, C1, C2, C3, lower, _spill_c3_to_src1
    from concourse.dve_uop import DveOpSpec

    for op in dve_ops.OPS:
        if op.name == "EXPB_ANT":
            return op
    a = Src0 + C0
    u = a - C0
    z = Src0 - u
    m2 = (z * C1 + C2) * z
    body = _spill_c3_to_src1((u + m2) + C3)
    spec = Spec(body=body, reference=_expb_ref)
    row = dve_ops._CUSTOM_DVE_ROW_BASE + len(dve_ops.OPS)
    dve_ops._SUB_OPCODE_FOR_NAME["EXPB_ANT"] = row
    shas = {}
    for ver in ("v3", "v4"):
        try:
            uops = lower(spec, ver=ver)
            shas[ver] = DveOpSpec(
                name="EXPB_ANT", opcode=row, uops=uops, rd1_en=True
            ).sha(ver)
        except Exception:
            pass
    op = dve_ops.DveOp("EXPB_ANT", spec, subdim=False, uops_sha=shas)
    dve_ops.OPS.append(op)
    dve_ops.CUSTOM_DVE_SPECS["EXPB_ANT"] = spec
    return op


def _emit(nc):
    # chunk-major inputs: per-partition fully contiguous DMA
    xT = nc.dram_tensor("xT", [128, NC8, PO, 512], BF16, kind="ExternalInput")
    xB = nc.dram_tensor("xB", [128, 4, PO, 512], BF16, kind="ExternalInput")
    wq = nc.dram_tensor("wq", [128, PO, 128], BF16, kind="ExternalInput")
    wk = nc.dram_tensor("wk", [128, PO, 128], BF16, kind="ExternalInput")
    wv = nc.dram_tensor("wv", [128, PO, 128], BF16, kind="ExternalInput")
    wp = nc.dram_tensor("wp", [64, 2, D], BF16, kind="ExternalInput")
    yTa = nc.dram_tensor("yTa", [128, PO, NC8, 512], BF16, kind="ExternalOutput")
    yTb = nc.dram_tensor("yTb", [128, PO, 4, 512], BF16, kind="ExternalOutput")

    with tile.TileContext(nc) as tc:
        with (
            tc.tile_pool(name="persist", bufs=1) as pp,
            tc.tile_pool(name="work", bufs=4) as wkp,
            tc.tile_pool(name="ps", bufs=2, space="PSUM") as psp,
        ):
            # ---------------- persistent SBUF tensors ----------------
            xT_sb = pp.tile([128, NC8, PO, 512], BF16)
            xB_sb = pp.tile([128, 4, PO, 512], BF16)
            wq_sb = pp.tile([128, PO, 128], BF16)
            wk_sb = pp.tile([128, PO, 128], BF16)
            wv_sb = pp.tile([128, PO, 128], BF16)
            wp_sb = pp.tile([64, 2, D], BF16)
            QT_sb = pp.tile([128, S], BF16)      # 0:64 A (full S); 64:128 B (cols 0:SU) + A-dup (cols 3072:4096)
            KT_sb = pp.tile([128, S], BF16)      # 0:64 A, 64:128 B
            KT2_sb = pp.tile([128, S], BF16)     # 64:128 = copy of A rows (for self-pairing)
            VT_sb = pp.tile([128, S], BF16)      # V^T staging for the DMA transpose
            V_sb = pp.tile([128, NT, 256], BF16)  # [V_A |ones| V_B |ones]
            ot_u0 = pp.tile([64, SU], BF16)
            ot_u1 = pp.tile([64, SU], BF16)
            ot_u2 = pp.tile([64, SU], BF16)
            ot_tiles = {"u0": ot_u0, "u1": ot_u1, "u2": ot_u2}

            # DMA order: block-0 critical inputs first, split across queues
            nc.sync.dma_start(wq_sb[:], wq.ap())
            nc.gpsimd.dma_start(wv_sb[:], wv.ap())
            nc.sync.dma_start(wk_sb[:], wk.ap())
            nc.gpsimd.dma_start(xB_sb[:, 0], xB.ap()[:, 0])
            nc.gpsimd.dma_start(wp_sb[:], wp.ap())
            for n in range(NC8):
                q = nc.sync if n % 2 == 0 else nc.gpsimd
                q.dma_start(xT_sb[:, n], xT.ap()[:, n])
                if n in (1, 2, 3):
                    nc.gpsimd.dma_start(xB_sb[:, n], xB.ap()[:, n])

            q0_sb = pp.tile([128, 1], F32)
            bias_sb = pp.tile([128, 1], F32)
            nc.vector.memset(V_sb[:, :, 64:128], 1.0)
            nc.vector.memset(V_sb[:, :, 192:256], 1.0)
            # HAM pre-warm: dense matmuls while the input DMAs stream so the
            # PE clock is at 2.4 GHz when real work starts
            warm_sb = pp.tile([128, 128], BF16)
            nc.vector.memset(warm_sb[:], 0.0)
            warm_ps = psp.tile([128, 512], F32, tag="aux", name="warm_ps")
            for i in range(64):
                nc.tensor.matmul(
                    warm_ps[:, 0:128], lhsT=warm_sb[:], rhs=warm_sb[:],
                    start=(i == 0), stop=(i == 63),
                )
            nc.vector.memset(q0_sb[:], EXP_Q0)
            nc.vector.memset(bias_sb[:], EXP_BIAS)
            expb = _expb_op()

            # ---------------- q/k/v projection blocks -----------------
            pfx_alt = [0]

            def pfx_copy(dst, srcv):
                pfx_alt[0] ^= 1
                if pfx_alt[0]:
                    nc.vector.tensor_copy(dst, srcv)
                else:
                    nc.scalar.copy(dst, srcv)

            def emit_qt_pair(n):
                # one PSUM bank: rows 0:64 <- head A cols n, rows 64:128 <-
                # head B (n<4, from xB = head-B query-row chunks) or head A
                # cols n+1 (n>=4)
                q_ps = psp.tile([128, 512], F32, tag="aux", name="q_ps")
                # two interleaved 6-chains sharing one bank via column tiling:
                # each chain has its own start/stop (the pending-zero clear is
                # per partition range)
                for po in range(PO):
                    nc.tensor.matmul(
                        q_ps[0:64, :],
                        lhsT=wq_sb[:, po, 0:64],
                        rhs=xT_sb[:, n, po, :],
                        start=(po == 0),
                        stop=(po == PO - 1),
                        skip_group_check=True,
                    )
                    if n < 4:
                        nc.tensor.matmul(
                            q_ps[64:128, :],
                            lhsT=wq_sb[:, po, 64:128],
                            rhs=xB_sb[:, n, po, :],
                            start=(po == 0),
                            stop=(po == PO - 1),
                            skip_group_check=True,
                        )
                    else:
                        nc.tensor.matmul(
                            q_ps[64:128, :],
                            lhsT=wq_sb[:, po, 0:64],
                            rhs=xT_sb[:, n + 1, po, :],
                            start=(po == 0),
                            stop=(po == PO - 1),
                            skip_group_check=True,
                        )
                if n < 4:
                    pfx_copy(QT_sb[:, ts(n, 512)], q_ps[:])
                else:
                    pfx_copy(QT_sb[0:64, ts(n, 512)], q_ps[0:64, :])
                    pfx_copy(QT_sb[0:64, ts(n + 1, 512)], q_ps[64:128, :])

            def emit_kt_block(n):
                k_ps = psp.tile([128, 512], F32, tag="aux", name="k_ps")
                for po in range(PO):
                    nc.tensor.matmul(
                        k_ps[:, 0:512],
                        lhsT=wk_sb[:, po, :],
                        rhs=xT_sb[:, n, po, :],
                        start=(po == 0),
                        stop=(po == PO - 1),
                    )
                pfx_copy(KT_sb[:, ts(n, 512)], k_ps[:, 0:512])

            def emit_vt_block(n):
                v_ps = psp.tile([128, 512], F32, tag="aux", name="v_ps")
                for po in range(PO):
                    nc.tensor.matmul(
                        v_ps[:, 0:512],
                        lhsT=wv_sb[:, po, :],
                        rhs=xT_sb[:, n, po, :],
                        start=(po == 0),
                        stop=(po == PO - 1),
                    )
                pfx_copy(VT_sb[:, ts(n, 512)], v_ps[:, 0:512])
                vv = V_sb[:, 4 * n : 4 * n + 4, :]
                nc.sync.dma_start_transpose(vv[:, :, 0:64], VT_sb[0:64, ts(n, 512)])
                nc.sync.dma_start_transpose(vv[:, :, 128:192], VT_sb[64:128, ts(n, 512)])

            # ---------------- attention sweeps ----------------
            # unit specs: (vbase, wp_idx, ydst, ycolbase)
            USPEC = {
                "u0": (0, 0, yTa, 0),
                "u1": (0, 0, yTa, SU),
                "u2": (128, 1, yTb, 0),
            }
            # sweeps: (L, R) sides: (unit, kt_tile, slot, qt_abs_col, ot_local_col)
            sweeps = []
            for sb in range(4):
                sweeps.append(
                    (
                        ("u0", KT_sb, 0, sb * 512, sb * 512),
                        ("u2", KT_sb, 64, sb * 512, sb * 512),
                    )
                )
            for sb in range(2):
                sweeps.append(
                    (
                        ("u1", KT_sb, 0, SU + sb * 512, sb * 512),
                        ("u1", KT2_sb, 64, SU + 1024 + sb * 512, 1024 + sb * 512),
                    )
                )

            proj_q = []
            cp_alt = [0]
            oq_alt = [0]

            tw_holder = []

            def emit_proj_chunk(u, oe, col, tail=False):
                _vb, wpi, ydst, ybase = USPEC[u]
                pj = psp.tile([128, 512], F32, tag="aux", name="pj")
                nc.tensor.matmul(
                    pj[:],
                    lhsT=wp_sb[:, wpi, ts(oe, 128)],
                    rhs=ot_tiles[u][0:64, ds(col, 512)],
                    start=True,
                    stop=True,
                )
                if tail:
                    # keep the PE dense so HAM stays at full clock; scL banks
                    # are idle in the tail
                    if not tw_holder:
                        tw_holder.append(
                            psp.tile([128, 512], F32, tag="scL", name="tw")
                        )
                    tw = tw_holder[0]
                    for _ in range(2):
                        nc.tensor.matmul(
                            tw[:, 0:128], lhsT=warm_sb[:], rhs=warm_sb[:],
                            start=True, stop=True,
                        )
                yt = wkp.tile([128, 512], BF16, tag="yt", name="yt", bufs=6)
                if tail:
                    cp_alt[0] ^= 1
                    if cp_alt[0]:
                        nc.scalar.copy(yt[:], pj[:])
                    else:
                        nc.vector.tensor_copy(yt[:], pj[:])
                else:
                    nc.scalar.copy(yt[:], pj[:])
                jc = (col + ybase) // 512
                oq_alt[0] ^= 1
                oq = nc.gpsimd if oq_alt[0] else nc.sync
                oq.dma_start(ydst.ap()[:, oe, jc], yt[:])

            def pump(k):
                for _ in range(k):
                    if proj_q:
                        proj_q.pop(0)()

            # pre-sweep: block-0 projections (just-in-time minimum)
            emit_qt_pair(0)
            emit_kt_block(0)
            emit_vt_block(0)

            for si, (Lside, Rside) in enumerate(sweeps):
                if si == 1:
                    # duplicates for head-A self-pairing (after KT/QT complete)
                    nc.gpsimd.dma_start(KT2_sb[64:128, :], KT_sb[0:64, :])
                    nc.gpsimd.dma_start(
                        QT_sb[64:128, SU + 1024 : S], QT_sb[0:64, SU + 1024 : S]
                    )
                avL = psp.tile([128, 512], F32, tag="av", name="avL")
                avR = psp.tile([128, 512], F32, tag="av", name="avR")

                def emit_av_pair(p0, p1):
                    # same-bank runs: avL(t0), avL(t1), avR(t0), avR(t1) —
                    # consecutive accumulations into one PSUM bank issue
                    # back-to-back with no bank-switch micro-idle
                    for idx, (u, av) in enumerate(((Lside[0], avL), (Rside[0], avR))):
                        vbase = USPEC[u][0]
                        for t, ptl, ptr in (p0, p1):
                            pt = ptl if idx == 0 else ptr
                            nc.tensor.matmul(
                                av,
                                lhsT=V_sb[:, t, vbase : vbase + 128],
                                rhs=pt[:, 0:512],
                                start=(t == 0),
                                stop=(t == NT - 1),
                            )

                pend = []
                for t in range(NT):
                    # qkv generation woven just-in-time into sweeps 0/1
                    if si == 0:
                        m = t // 4
                        if t % 4 == 0 and m < 7:
                            emit_kt_block(m + 1)
                        elif t % 4 == 1 and m < 7:
                            emit_vt_block(m + 1)
                        elif t == 2:
                            emit_qt_pair(1)
                        elif t == 22:
                            emit_qt_pair(4)
                        elif t == 26:
                            emit_qt_pair(6)
                    elif si == 1:
                        if t == 0:
                            emit_qt_pair(2)
                        elif t == 4:
                            emit_qt_pair(3)
                    scL = psp.tile([128, 512], F32, tag="scL", name="scL")
                    scR = psp.tile([128, 512], F32, tag="scR", name="scR")
                    for (u, ktt, slot, qcol, _ocol), sct in ((Lside, scL), (Rside, scR)):
                        nc.tensor.matmul(
                            sct[:, 0:512],
                            lhsT=ktt[slot : slot + 64, ts(t, 128)],
                            rhs=QT_sb[slot : slot + 64, ds(qcol, 512)],
                            start=True,
                            stop=True,
                        )
                    ptL = wkp.tile([128, 512], BF16, tag="ptL", name="ptL", bufs=6)
                    ptR = wkp.tile([128, 512], BF16, tag="ptR", name="ptR", bufs=6)
                    nc.scalar.activation(
                        ptL[:, 0:512],
                        scL[:, 0:512],
                        mybir.ActivationFunctionType.Exp,
                        bias=bias_sb[:],
                        scale=EXP_SCALE,
                    )
                    nc.vector._custom_dve(
                        expb,
                        out=ptR[:, 0:512].bitcast(mybir.dt.uint16),
                        in0=scR[:, 0:512],
                        in1=q0_sb[:],
                        s0=EXP_M,
                        s1=EXP_Q2,
                        imm2=EXP_Q1,
                    )
                    pend.append((t, ptL, ptR))
                    if t % 2 == 1:
                        # AVs lag two chunk-pairs behind the scores so their
                        # exp inputs are done at issue: every PE instruction is
                        # dep-free (throughput-bound, not exp-latency-bound)
                        if len(pend) >= 6:
                            emit_av_pair(pend.pop(0), pend.pop(0))
                        pump(1)
                while len(pend) >= 2:
                    emit_av_pair(pend.pop(0), pend.pop(0))
                # drain sweeps: normalize O^T, queue proj work
                for (u, _ktt, _slot, _qcol, ocol), av in ((Lside, avL), (Rside, avR)):
                    rhi = wkp.tile([128, 512], F32, tag="rhi", name="rhi", bufs=3)
                    rlo = wkp.tile([64, 512], F32, tag="rlo", name="rlo", bufs=3)
                    # custom-DVE ops require base_partition 0: run on the full
                    # tile; rows 0:64 (1/O^T) are computed but never read
                    nc.vector.reciprocal_approx_fast(rhi[:], av[:])
                    nc.sync.dma_start(rlo[:], rhi[64:128, :])
                    nc.vector.tensor_mul(
                        ot_tiles[u][0:64, ds(ocol, 512)], av[0:64, :], rlo[:]
                    )
                    for oe in range(PO):
                        proj_q.append(
                            lambda tail=False, u=u, oe=oe, col=ocol: emit_proj_chunk(
                                u, oe, col, tail
                            )
                        )
            while proj_q:
                proj_q.pop(0)(tail=True)

    nc.compile()
    return nc


def _build():
    if "nc" not in _CACHE:
        nc = bacc.Bacc(None, target_bir_lowering=False, debug=False)
        _CACHE["nc"] = _emit(nc)
    return _CACHE["nc"]


def _prep_inputs(x, w_qkv, w_proj):
    bf = ml_dtypes.bfloat16
    xs = np.ascontiguousarray(x.reshape(S, D).T).astype(bf)  # [D, S]
    # chunk-major [pi, n, po, s]: element = xT[po*128+pi, n*512+s]
    x8 = np.ascontiguousarray(
        xs.reshape(PO, 128, NC8, 512).transpose(1, 2, 0, 3)
    )
    in_maps = []
    for c in range(NCORES):
        ha = c
        hb = 8 + c // 2
        bh = c % 2
        rows_q = lambda h: w_qkv[h * HD : (h + 1) * HD, :]
        rows_k = lambda h: w_qkv[D + h * HD : D + (h + 1) * HD, :]
        rows_v = lambda h: w_qkv[2 * D + h * HD : 2 * D + (h + 1) * HD, :]
        qs = SCALE * LOG2E_128
        wq_c = np.concatenate([rows_q(ha) * qs, rows_q(hb) * qs], 0).T  # [D, 128]
        wk_c = np.concatenate([rows_k(ha), rows_k(hb)], 0).T
        wv_c = np.concatenate([rows_v(ha), rows_v(hb)], 0).T
        wp_c = np.stack(
            [w_proj[:, ha * HD : (ha + 1) * HD].T, w_proj[:, hb * HD : (hb + 1) * HD].T],
            axis=1,
        )  # [64, 2, D]
        shuf = lambda w: np.ascontiguousarray(
            w.reshape(PO, 128, 128).transpose(1, 0, 2)
        ).astype(bf)
        in_maps.append(
            {
                "xT": x8,
                "xB": np.ascontiguousarray(x8[:, bh * 4 : bh * 4 + 4]),
                "wq": shuf(wq_c),
                "wk": shuf(wk_c),
                "wv": shuf(wv_c),
                "wp": np.ascontiguousarray(wp_c).astype(bf),
            }
        )
    return in_maps


def _combine(results, b_proj):
    yT = np.zeros((D, S), np.float32)
    for c in range(NCORES):
        ya = results[c]["yTa"].astype(np.float32)  # [128, 6, 8, 512]
        yT += ya.transpose(1, 0, 2, 3).reshape(D, S)
        bh = c % 2
        yb = results[c]["yTb"].astype(np.float32)  # [128, 6, 4, 512]
        yT[:, bh * SU : (bh + 1) * SU] += yb.transpose(1, 0, 2, 3).reshape(D, SU)
    y = yT.T + b_proj.astype(np.float32)[None, :]
    return y.reshape(1, 64, 64, D).astype(np.float32)


def kernel(x, w_qkv, w_proj, b_proj, _trace=False, _trace_kwargs=None):
    x = np.asarray(x, np.float32)
    w_qkv = np.asarray(w_qkv, np.float32)
    w_proj = np.asarray(w_proj, np.float32)
    b_proj = np.asarray(b_proj, np.float32)

    nc = _build()
    in_maps = _prep_inputs(x, w_qkv, w_proj)
    res = run_bass_kernel_spmd(
        nc, in_maps, core_ids=list(range(NCORES)), trace=_trace,
        **(_trace_kwargs or {}),
    )
    out = _combine(res.results, b_proj)
    if _trace:
        return out, res
    return out
